# revision 1
# baseline (speedup 1.0000x reference)
import sys as _sys
for _p in ("/opt/trn_rl_repo", "/opt/pypackages"):
    if _p not in _sys.path:
        _sys.path.insert(0, _p)
"""GATv2 message-passing kernel for TRN2 (Bass/Tile), data-parallel over dst-node ranges.

Design:
  - Host folds BatchNorm into W_l/W_r, sorts edges by dst, partitions nodes/edges
    across cores (contiguous dst ranges), groups edges by 128-node dst groups,
    pads each group's edge count to a multiple of 128 (chunks).
  - Device, per 128-edge chunk:
      * indirect-gather x[src] rows (bf16) from the full x table in HBM
      * sequential load of host-pre-transposed edge_attr chunk (bf16)
      * one-hot M[e,n] = (dstloc_e == n) built via DVE is_equal against an iota
      * PE: m = x_src@W_l + ea@W_e + M^T-expand(x_grp@W_r + bias)  (PSUM accumulate)
            xl = x_src@W_l kept separately for the value path
      * leaky-relu (max(x, 0.2x)), per-head att dot (DVE mul+reduce), exp on ACT
        (broadcast back to [H,C]), v = exp(alpha) * xl
      * scatter: PSUM += M.T @ [v | s]  (f32r matmuls; exact for one-hot M)
  - Per group: normalize by segment sums, head-mean, +bias, relu, and pool into
    a per-graph PSUM accumulator via another one-hot matmul.
  - Per core output: [G, 2] partial of pooled@W_lin / cnt; host sums cores + b_lin.
"""

import math
from contextlib import ExitStack
from dataclasses import dataclass, field

import numpy as np
import ml_dtypes

import concourse.bacc as bacc
import concourse.tile as tile
from concourse import bass, mybir
from concourse.masks import make_identity

F32 = mybir.dt.float32
F32R = mybir.dt.float32r
BF16 = mybir.dt.bfloat16
I32 = mybir.dt.int32

BN_EPS = 1e-5
NEG_SLOPE = 0.2
PAD_SENTINEL = 200.0  # one-hot compare value that never matches (> 127)


@dataclass
class Cfg:
    N: int
    E: int
    G: int
    n_cores: int
    F: int = 128
    H: int = 10
    C: int = 64
    Kg: list = field(default_factory=list)  # chunks per group (shared across cores)
    debug: bool = False
    use_lrelu: bool = False   # HW Lrelu activation instead of max(x, 0.2x)
    psum_add: bool = False    # DVE add of xl_ps+m_ps instead of double xl matmul
    scat_bf16: bool = False   # scatter matmuls in bf16 instead of f32r
    xl_in_m: bool = True      # xl computed in m_ps then drained (HW-only pattern)

    @property
    def HC(self):
        return self.H * self.C

    @property
    def NPC(self):
        assert self.N % self.n_cores == 0
        return self.N // self.n_cores

    @property
    def GPC(self):
        return (self.NPC + 127) // 128

    @property
    def TOTCH(self):
        return sum(self.Kg)


def fold_bn(inp):
    """Fold BatchNorm into the linear weights. Returns fp32 arrays."""
    g = np.float64(inp["bn_weight"]) / np.sqrt(np.float64(inp["bn_var"]) + BN_EPS)
    c0 = np.float64(inp["bn_bias"]) - np.float64(inp["bn_mean"]) * g
    Wl = g[:, None] * np.float64(inp["W_l"])
    Wr = g[:, None] * np.float64(inp["W_r"])
    bl = np.float64(inp["b_l"]) + c0 @ np.float64(inp["W_l"])
    br = np.float64(inp["b_r"]) + c0 @ np.float64(inp["W_r"])
    return (Wl.astype(np.float32), Wr.astype(np.float32),
            (bl + br).astype(np.float32), bl.astype(np.float32))


def preprocess(inp, n_cores, G):
    """Host-side sharding. Returns (cfg, in_maps, b_lin)."""
    x = np.asarray(inp["x"], np.float32)
    ea = np.asarray(inp["edge_attr"], np.float32)
    edge_index = np.asarray(inp["edge_index"], np.int64)
    batch = np.asarray(inp["batch"], np.int64)
    N, F = x.shape
    E = edge_index.shape[1]

    cfg = Cfg(N=N, E=E, G=G, n_cores=n_cores, F=F)
    NPC, GPC = cfg.NPC, cfg.GPC

    Wl, Wr, bsum, bl_eff = fold_bn(inp)
    att = np.asarray(inp["att"], np.float32).reshape(-1)  # [H*C]
    We = np.asarray(inp["W_e"], np.float32)
    bias = np.asarray(inp["bias"], np.float32)
    W_lin = np.asarray(inp["W_lin"], np.float32)
    b_lin = np.asarray(inp["b_lin"], np.float32)
    H, C, HC = cfg.H, cfg.C, cfg.HC
    assert HC == Wl.shape[1]

    src = edge_index[0].astype(np.int64)
    dst = edge_index[1].astype(np.int64)

    # --- partition edges by (core, group) and compute per-(core,group) chunk counts
    core_of = dst // NPC
    grp_of = (dst % NPC) // 128
    # edge ids per (core, group), dst-major stable order
    order = np.lexsort((np.arange(E), dst))
    counts = np.zeros((n_cores, GPC), np.int64)
    np.add.at(counts, (core_of, grp_of), 1)
    Kg = np.maximum(1, np.ceil(counts / 128.0).astype(np.int64).max(axis=0))
    cfg.Kg = [int(k) for k in Kg]
    TOTCH = cfg.TOTCH
    chunk_base = np.concatenate([[0], np.cumsum(Kg)])  # per-group chunk offsets

    ea_bf = ea.astype(ml_dtypes.bfloat16)
    x_bf = x.astype(ml_dtypes.bfloat16)

    cnt = np.bincount(batch, minlength=G).astype(np.float32)
    cinv = (1.0 / np.maximum(cnt, 1.0)).reshape(G, 1).astype(np.float32)

    # shared consts. Weights are padded with 10 extra columns holding the
    # att-projection of each weight block scaled by the leaky slope:
    # lrelu(m) = slope*m + (1-slope)*relu(m), and att.(slope*m) is linear in m,
    # so the m-matmuls compute it directly into columns HC:HC+H.
    attm = att.reshape(H, C)  # [H, C]
    def pad_att(W):
        Wp = np.zeros((F, HC + H), np.float64)
        Wp[:, :HC] = W
        for h in range(H):
            Wp[:, HC + h] = NEG_SLOPE * (W[:, h * C:(h + 1) * C] @ attm[h])
        return Wp.astype(ml_dtypes.bfloat16)
    wl_b = pad_att(np.float64(Wl))
    wr_b = pad_att(np.float64(Wr))
    we_b = pad_att(np.float64(We))
    # att multiplier for the relu branch carries the (1-slope) factor
    attb = np.broadcast_to(((1.0 - NEG_SLOPE) * att).astype(ml_dtypes.bfloat16), (128, HC)).copy()
    bsum_att = np.concatenate([bsum, NEG_SLOPE * (bsum.reshape(H, C) * attm).sum(axis=1)])
    bsumb = np.broadcast_to(bsum_att.astype(np.float32), (128, HC + H)).copy()
    # value-path b_l enters after softmax (weights sum to 1): fold its head-mean
    # into the output bias (exact for nodes with >=1 in-edge)
    bias_eff = bias + bl_eff.reshape(H, C).mean(axis=0)
    biasb = np.broadcast_to(bias_eff, (128, C)).copy().astype(np.float32)

    # per-core edge id layout [TOTCH*128], -1 = pad
    sorted_eids = order  # edge ids sorted by dst
    sorted_core = core_of[order]
    sorted_grp = grp_of[order]

    in_maps = []
    for c in range(n_cores):
        sel = sorted_core == c
        eids_c = sorted_eids[sel]
        grp_c = sorted_grp[sel]
        slot = np.full(TOTCH * 128, -1, np.int64)
        for g in range(GPC):
            ge = eids_c[grp_c == g]
            base = chunk_base[g] * 128
            slot[base:base + len(ge)] = ge
        pad = slot < 0
        eidx = np.where(pad, 0, slot)

        srci = src[eidx].astype(np.int32)
        srci[pad] = 0
        srci = srci.reshape(TOTCH, 128).T.copy()  # [128, TOTCH]

        # one-hot matrices, precomputed: M_f[e, n] = (dstloc_e == n) as f32 for
        # the f32r scatter lhsT; MT_b[n, e] transposed bf16 for the expand lhsT
        gidx = np.repeat(np.arange(TOTCH), 128)
        g_of_chunk = np.searchsorted(chunk_base[1:], gidx, side="right")
        dstl = (dst[eidx] % NPC - g_of_chunk * 128).astype(np.int64)
        dstl[pad] = 10**6
        dstl2 = dstl.reshape(TOTCH, 128)
        onehot = (dstl2[:, :, None] == np.arange(128)[None, None, :])  # [T, e, n]
        m_f = onehot.astype(np.float32).reshape(TOTCH * 128, 128)
        mt_b = onehot.transpose(0, 2, 1).astype(ml_dtypes.bfloat16).reshape(TOTCH * 128, 128)

        eat = ea_bf[eidx]  # [TOTCH*128, F]
        eat[pad] = 0
        eat = eat.reshape(TOTCH, 128, F).transpose(0, 2, 1).reshape(TOTCH * F, 128).copy()

        xo = np.zeros((GPC * 128, F), ml_dtypes.bfloat16)
        xo[:NPC] = x_bf[c * NPC:(c + 1) * NPC]

        nodes = c * NPC + np.arange(GPC * 128)
        bl = np.where(nodes < min(N, (c + 1) * NPC), batch[np.minimum(nodes, N - 1)], int(PAD_SENTINEL))
        bloc = bl.reshape(GPC, 128).T.copy().astype(np.float32)  # [128, GPC]

        in_maps.append({
            "xtab": x_bf, "xown": xo, "eat": eat,
            "srci": srci, "mf": m_f, "mtb": mt_b, "bloc": bloc,
            "wl": wl_b, "wr": wr_b, "we": we_b,
            "attb": attb, "bsumb": bsumb, "biasb": biasb,
            "wlin": W_lin, "cinv": cinv,
        })
    return cfg, in_maps, b_lin


def build_kernel(cfg: Cfg):
    H, C, HC, F, G = cfg.H, cfg.C, cfg.HC, cfg.F, cfg.G
    GPC, Kg, TOTCH = cfg.GPC, cfg.Kg, cfg.TOTCH
    EQ = mybir.AluOpType.is_equal
    ADD = mybir.AluOpType.add
    MULT = mybir.AluOpType.mult
    MAX = mybir.AluOpType.max
    AX = mybir.AxisListType.X
    ACT = mybir.ActivationFunctionType

    nc = bacc.Bacc("TRN2", target_bir_lowering=False, debug=cfg.debug,
                   num_devices=cfg.n_cores)
    xtab = nc.dram_tensor("xtab", [cfg.N, F], BF16, kind="ExternalInput")
    xown = nc.dram_tensor("xown", [GPC * 128, F], BF16, kind="ExternalInput")
    eat = nc.dram_tensor("eat", [TOTCH * F, 128], BF16, kind="ExternalInput")
    srci = nc.dram_tensor("srci", [128, TOTCH], I32, kind="ExternalInput")
    mf_d = nc.dram_tensor("mf", [TOTCH * 128, 128], F32R, kind="ExternalInput")
    mtb_d = nc.dram_tensor("mtb", [TOTCH * 128, 128], BF16, kind="ExternalInput")
    bloc = nc.dram_tensor("bloc", [128, GPC], F32, kind="ExternalInput")
    wl_d = nc.dram_tensor("wl", [F, HC + H], BF16, kind="ExternalInput")
    wr_d = nc.dram_tensor("wr", [F, HC + H], BF16, kind="ExternalInput")
    we_d = nc.dram_tensor("we", [F, HC + H], BF16, kind="ExternalInput")
    attb_d = nc.dram_tensor("attb", [128, HC], BF16, kind="ExternalInput")
    bsumb_d = nc.dram_tensor("bsumb", [128, HC + H], F32, kind="ExternalInput")
    biasb_d = nc.dram_tensor("biasb", [128, C], F32, kind="ExternalInput")
    wlin_d = nc.dram_tensor("wlin", [C, 2], F32, kind="ExternalInput")
    cinv_d = nc.dram_tensor("cinv", [G, 1], F32, kind="ExternalInput")
    out_d = nc.dram_tensor("out", [G, 2], F32, kind="ExternalOutput")

    with tile.TileContext(nc) as tc, ExitStack() as ctx:
        cp = ctx.enter_context(tc.tile_pool(name="const", bufs=1))
        sp = ctx.enter_context(tc.tile_pool(name="small", bufs=4))
        bp = ctx.enter_context(tc.tile_pool(name="big", bufs=4))
        pp = ctx.enter_context(tc.tile_pool(name="ps", bufs=1, space="PSUM"))
        ppm = ctx.enter_context(tc.tile_pool(name="psm", bufs=2 if cfg.xl_in_m else 1, space="PSUM"))
        ppt = ctx.enter_context(tc.tile_pool(name="pst", bufs=2, space="PSUM"))

        def cload(name, dram, shape, dt):
            t = cp.tile(shape, dt, tag=name)
            nc.sync.dma_start(t[:], dram.ap())
            return t

        wl = cload("wl", wl_d, [F, HC + H], BF16)
        wr = cload("wr", wr_d, [F, HC + H], BF16)
        we = cload("we", we_d, [F, HC + H], BF16)
        attb = cload("attb", attb_d, [128, HC], BF16)
        bsumb = cload("bsumb", bsumb_d, [128, HC + H], F32)
        biasb = cload("biasb", biasb_d, [128, C], F32)
        wlin = cload("wlin", wlin_d, [C, 2], F32)
        cinv = cload("cinv", cinv_d, [G, 1], F32)
        srcs = cload("srcs", srci, [128, TOTCH], I32)
        blocs = cload("blocs", bloc, [128, GPC], F32)

        ident = cp.tile([128, 128], BF16, tag="ident")
        make_identity(nc, ident[:])
        iotaF = cp.tile([128, 128], F32, tag="iotaF")
        nc.gpsimd.iota(iotaF[:], pattern=[[1, 128]], base=0, channel_multiplier=0,
                       allow_small_or_imprecise_dtypes=True)

        poolacc = cp.tile([C, G], F32, tag="poolacc")
        nc.gpsimd.memset(poolacc[:], 0.0)

        NSPL = [(0, 512), (512, HC)]
        NSPLA = [(0, 512), (512, HC + H)]
        t0 = 0
        for g in range(GPC):
            # group-level: xr = xown_group @ Wr + bsum
            xg = sp.tile([128, F], BF16, tag="xg")
            nc.sync.dma_start(xg[:], xown.ap()[g * 128:(g + 1) * 128, :])
            xgT_ps = ppt.tile([128, 128], BF16, tag="tp")
            nc.tensor.transpose(xgT_ps[:], xg[:], ident[:])
            xgT = sp.tile([128, 128], BF16, tag="xgT")
            nc.scalar.copy(xgT[:], xgT_ps[:])
            xr_ps = ppm.tile([128, HC + H], F32, tag="m")
            for a, b in NSPLA:
                nc.tensor.matmul(xr_ps[:, a:b], lhsT=xgT[:], rhs=wr[:, a:b],
                                 start=True, stop=True)
            xr = bp.tile([128, HC + H], BF16, tag="xr")
            nc.vector.tensor_tensor(out=xr[:], in0=xr_ps[:], in1=bsumb[:], op=ADD)

            scat = pp.tile([128, HC + H], F32, tag="scat")
            for k in range(Kg[g]):
                t = t0 + k
                first, last = k == 0, k == Kg[g] - 1
                xn = sp.tile([128, F], BF16, tag="xn")
                nc.gpsimd.indirect_dma_start(
                    out=xn[:], out_offset=None, in_=xtab.ap(),
                    in_offset=bass.IndirectOffsetOnAxis(ap=srcs[:, t:t + 1], axis=0))
                eat_t = sp.tile([F, 128], BF16, tag="eat_t")
                nc.scalar.dma_start(eat_t[:], eat.ap()[t * F:(t + 1) * F, :])
                M_f = sp.tile([128, 128], F32R, tag="M_f")
                nc.sync.dma_start(M_f[:], mf_d.ap()[t * 128:(t + 1) * 128, :])
                MT = sp.tile([128, 128], BF16, tag="MT")
                nc.sync.dma_start(MT[:], mtb_d.ap()[t * 128:(t + 1) * 128, :])
                xnT_ps = ppt.tile([128, 128], BF16, tag="tp")
                nc.tensor.transpose(xnT_ps[:], xn[:], ident[:])
                xnT = sp.tile([128, 128], BF16, tag="xnT")
                nc.scalar.copy(xnT[:], xnT_ps[:])

                m_ps = ppm.tile([128, HC + H], F32, tag="m")
                if cfg.xl_in_m:
                    for a, b in NSPLA:
                        nc.tensor.matmul(m_ps[:, a:b], lhsT=xnT[:], rhs=wl[:, a:b],
                                         start=True, stop=True)
                    xl_f = bp.tile([128, HC], F32, tag="xl_f")
                    nc.scalar.copy(xl_f[:], m_ps[:, 0:HC])
                    for a, b in NSPLA:
                        nc.tensor.matmul(m_ps[:, a:b], lhsT=eat_t[:], rhs=we[:, a:b],
                                         start=False, stop=False, skip_group_check=True)
                    for a, b in NSPLA:
                        nc.tensor.matmul(m_ps[:, a:b], lhsT=MT[:], rhs=xr[:, a:b],
                                         start=False, stop=True)
                else:
                    xl_ps = pp.tile([128, HC], F32, tag="xl")
                    for a, b in NSPLA:
                        nc.tensor.matmul(m_ps[:, a:b], lhsT=xnT[:], rhs=wl[:, a:b],
                                         start=True, stop=False)
                    for a, b in NSPLA:
                        nc.tensor.matmul(m_ps[:, a:b], lhsT=eat_t[:], rhs=we[:, a:b],
                                         start=False, stop=False)
                    for a, b in NSPLA:
                        nc.tensor.matmul(m_ps[:, a:b], lhsT=MT[:], rhs=xr[:, a:b],
                                         start=False, stop=True)
                    for a, b in NSPL:
                        nc.tensor.matmul(xl_ps[:, a:b], lhsT=xnT[:], rhs=wl[:, a:b],
                                         start=True, stop=True)

                mrelu = bp.tile([128, HC], BF16, tag="mrelu")
                nc.scalar.activation(mrelu[:], m_ps[:, 0:HC], ACT.Relu)
                a2 = sp.tile([128, H], F32, tag="a2")
                nc.scalar.copy(a2[:], m_ps[:, HC:HC + H])
                prod = bp.tile([128, HC], BF16, tag="prod")
                nc.vector.tensor_tensor(out=prod[:], in0=mrelu[:], in1=attb[:], op=MULT)
                ar = sp.tile([128, H], F32, tag="ar")
                nc.vector.tensor_reduce(out=ar[:],
                                        in_=prod[:].rearrange("p (h c) -> p h c", h=H),
                                        axis=AX, op=ADD)
                al = sp.tile([128, H], F32, tag="al")
                nc.vector.tensor_tensor(out=al[:], in0=a2[:], in1=ar[:], op=ADD)
                v = bp.tile([128, HC + H], BF16 if cfg.scat_bf16 else F32R, tag="v")
                nc.scalar.activation(v[:, HC:HC + H], al[:], ACT.Exp)
                vin = xl_f if cfg.xl_in_m else xl_ps
                nc.vector.tensor_tensor(out=v[:, 0:HC].rearrange("p (h c) -> p h c", h=H),
                                        in0=vin[:].rearrange("p (h c) -> p h c", h=H),
                                        in1=v[:, HC:HC + H].to_broadcast([128, H, C]),
                                        op=MULT)

                nc.tensor.matmul(scat[:, 0:512], lhsT=M_f[:],
                                 rhs=v[:, 0:512], start=first, stop=last)
                nc.tensor.matmul(scat[:, 512:HC + H], lhsT=M_f[:],
                                 rhs=v[:, 512:HC + H], start=first, stop=last)
            t0 += Kg[g]

            # group postprocess
            d10 = sp.tile([128, H], F32, tag="d10")
            nc.vector.tensor_scalar(out=d10[:], in0=scat[:, HC:HC + H],
                                    scalar1=1e-16, scalar2=float(H), op0=ADD, op1=MULT)
            rec = sp.tile([128, H], F32, tag="rec")
            nc.vector.reciprocal(rec[:], d10[:])
            osc = bp.tile([128, HC], F32, tag="osc")
            nc.vector.tensor_tensor(out=osc[:].rearrange("p (h c) -> p h c", h=H),
                                    in0=scat[:, 0:HC].rearrange("p (h c) -> p h c", h=H),
                                    in1=rec[:].to_broadcast([128, H, C]), op=MULT)
            red = sp.tile([128, C], F32, tag="red")
            nc.vector.tensor_reduce(out=red[:],
                                    in_=osc[:].rearrange("p (h c) -> p c h", h=H),
                                    axis=AX, op=ADD)
            rb = sp.tile([128, C], F32, tag="rb")
            nc.vector.tensor_tensor(out=rb[:], in0=red[:], in1=biasb[:], op=ADD)
            og = sp.tile([128, C], BF16, tag="og")
            nc.scalar.activation(og[:], rb[:], ACT.Relu)
            oh = sp.tile([128, G], BF16, tag="oh")
            nc.vector.tensor_scalar(out=oh[:], in0=iotaF[:, :G],
                                    scalar1=blocs[:, g:g + 1], scalar2=None, op0=EQ)
            pool_ps = ppt.tile([C, G], F32, tag="tp")
            nc.tensor.matmul(pool_ps[:], lhsT=og[:], rhs=oh[:], start=True, stop=True)
            nc.vector.tensor_tensor(out=poolacc[:], in0=pool_ps[:], in1=poolacc[:], op=ADD)

        poolT = poolacc
        fin_ps = ppt.tile([G, 2], F32, tag="tp")
        nc.tensor.matmul(fin_ps[:], lhsT=poolT[:], rhs=wlin[:], start=True, stop=True)
        fin = sp.tile([G, 2], F32, tag="fin")
        nc.vector.tensor_scalar(out=fin[:], in0=fin_ps[:], scalar1=cinv[:, :1],
                                scalar2=None, op0=MULT)
        nc.sync.dma_start(out_d.ap(), fin[:])

    nc.compile()
    return nc


def postprocess(core_outs, b_lin):
    return np.sum(np.stack(core_outs), axis=0).astype(np.float32) + b_lin


# ---------------------------------------------------------------------------
# Self-contained entry point: kernel(**inputs) -> np.ndarray [G, 2]
# ---------------------------------------------------------------------------
_G_GRAPHS = 64
_N_CORES = 8


def kernel(**inputs):
    import numpy as _np
    inp = {k: _np.asarray(v) for k, v in inputs.items()}
    cfg, in_maps, b_lin = preprocess(inp, _N_CORES, _G_GRAPHS)
    nc = build_kernel(cfg)
    from concourse.bass_utils import run_bass_kernel_spmd
    res = run_bass_kernel_spmd(nc, in_maps, list(range(_N_CORES)), trace=False)
    outs = [res.results[c]["out"] for c in range(_N_CORES)]
    return postprocess(outs, b_lin)

